# revision 2
# baseline (speedup 1.0000x reference)
"""Trainium2 Bass kernel for masked bi-linear attention.

Computes, for full inputs
    k:    [B, KL, E] f32
    q:    [B, Q,  E] f32
    W:    [E, E]     f32
    mask: [B, Q, KL] i32 (0/1)
the reference
    qw    = q @ W                      [B, Q, E]
    s     = qw @ k^T                   [B, Q, KL]
    p     = softmax(s, axis=-1) * mask
    out   = p @ k                      [B, Q, E]

Sharding: data-parallel over B across 8 NeuronCores (2 batches/core),
W replicated. Each core runs the same Bass program on its B-slice.

Precision: q/W/k in fp16 (11-bit mantissa) for all three matmuls;
softmax in fp32 on ACT/DVE; probabilities (in [0,1]) in fp16 for p @ k.

The PE is the bottleneck (~90% occupancy): 2560 N=512 matmuls/core
stream at 1 col/cycle.  All transposes (q^T, k^T, p^T) run on the DMA
xbar (dma_start_transpose, fp16) so the PE does only matmuls:
  q:  HBM -> qin f32 -> [DVE] q16 fp16 -> [xbar] qT [P, EC, QB]
  k:  HBM -> kin f32 -> [DVE] k16 [P, KC, E] fp16 (also the PV moving
      operand; double-buffered) -> [xbar] kTh [P, EC, KL]
  p:  exp+mask -> spb fp16 -> [xbar] pT [P, KC, P]

Pipelining, per q-tile t: scores(t) [PE] -> row-max [DVE] -> pT
transpose of t-1 [xbar] -> exp+mask (t) [ACT/DVE] -> PV matmuls of t-2
[PE] -> 1/z scale + output DMA.  Mask DMAs prefetch at tile start, q
DMAs/casts two blocks ahead (also across the batch boundary), and the
next batch's k chunks stream during the previous batch's last two
blocks so the batch hand-off costs only the 16 kTh xbar transposes
(overlapped with block-0 qw + the two deferred PVs).
"""

import numpy as np

import concourse.bacc as bacc
import concourse.mybir as mybir
import concourse.tile as tile
from concourse.bass_utils import run_bass_kernel_spmd
from contextlib import ExitStack

dt = mybir.dt
AF = mybir.ActivationFunctionType
ALU = mybir.AluOpType
AX = mybir.AxisListType

P = 128

N_CORES = 8
B, Q_LEN, K_LEN, EMB = 16, 2048, 2048, 1024


def emit_attention(ctx, tc, k_ap, q_ap, w_ap, mask_ap, out_ap,
                   Bl, Q, KL, E, QB=512):
    """Emit the per-core attention program.

    k_ap [Bl, KL, E], q_ap [Bl, Q, E], w_ap [E, E], mask_ap [Bl, Q, KL],
    out_ap [Bl, Q, E].
    """
    nc = tc.nc
    f32, f16, i32 = dt.float32, dt.float16, dt.int32

    assert Q % QB == 0 and QB % P == 0 and KL % P == 0 and E % P == 0
    EC = E // P          # e (contraction for qw) chunks
    KC = KL // P         # k chunks
    FC = E // P          # f chunks (qw output tiles)
    nqb = Q // QB
    qt_per_b = QB // P
    KB = min(512, KL)    # score psum block (<= 1 bank)
    nkb = KL // KB
    EB = min(512, E)     # PV psum block
    neb = E // EB

    big = ctx.enter_context(tc.tile_pool(name="big", bufs=1))
    qio = ctx.enter_context(tc.tile_pool(name="qio", bufs=5))
    q16p = ctx.enter_context(tc.tile_pool(name="q16p", bufs=3))
    mio = ctx.enter_context(tc.tile_pool(name="mio", bufs=2))
    ptp = ctx.enter_context(tc.tile_pool(name="ptp", bufs=4))
    work = ctx.enter_context(tc.tile_pool(name="work", bufs=2))
    small = ctx.enter_context(tc.tile_pool(name="small", bufs=3))
    psum = ctx.enter_context(tc.tile_pool(name="psum", bufs=4, space="PSUM"))
    psum_o = ctx.enter_context(tc.tile_pool(name="psum_o", bufs=2, space="PSUM"))

    # ---- W: loaded once per core as fp16
    wH = big.tile([P, EC * E], f16, tag="wH")

    def emit_w_load():
        for ec in range(EC):
            win = qio.tile([P, E], f32, tag="qin", name="win")
            nc.sync.dma_start(win[:], w_ap[ec * P:(ec + 1) * P, :])
            nc.scalar.copy(wH[:, ec * E:(ec + 1) * E], win[:])

    # ---- k pipeline: kin f32 DMA -> k16 fp16 (DVE cast); the fp16
    # natural-layout tensor is both the PV moving operand and the xbar
    # transpose source for kTh
    k16_of = {}

    def start_k(b):
        t = big.tile([P, KC, E], f16, tag="k16", name="k16", bufs=2)
        k16_of[b] = t
        return t

    def emit_k_chunks(b, kcs):
        t = k16_of[b]
        for kc in kcs:
            kin = qio.tile([P, E], f32, tag="qin", name="kin")
            nc.sync.dma_start(kin[:], k_ap[b, kc * P:(kc + 1) * P, :])
            nc.vector.tensor_copy(t[:, kc, :], kin[:])

    def emit_kTh(b):
        kTh = big.tile([P, EC, KL], f16, tag="kTh", name="kTh")
        t = k16_of[b]
        for kc in range(KC):
            nc.sync.dma_start_transpose(
                kTh[:, :, kc * P:(kc + 1) * P], t[:, kc, :])
        return kTh

    # deferred-PV state: (b, row0, spb, rz, k16t, [pT])
    pending = []

    def pv_prep(st):
        # p^T via the DMA xbar (SBUF->SBUF, fp16) on the sync queue
        spb = st[2]
        pT = ptp.tile([P, KC, P], f16, tag="pT", name="pT", bufs=2)
        nc.sync.dma_start_transpose(pT[:], spb[:])
        st.append(pT)
        return pT

    def pv_mms(st, pT):
        b, row0, spb, rz, k16t = st[:5]
        po = [psum_o.tile([P, EB], f32, tag=f"po{eh}", name=f"po{eh}")
              for eh in range(neb)]
        for kc in range(KC):
            for eh in range(neb):
                nc.tensor.matmul(
                    po[eh][:], pT[:, kc, :],
                    k16t[:, kc, eh * EB:(eh + 1) * EB],
                    start=(kc == 0), stop=(kc == KC - 1))
        for eh in range(neb):
            ot = mio.tile([P, EB], f32, tag="ot", name="ot")
            nc.scalar.activation(ot[:], po[eh][:], AF.Copy, scale=rz[:])
            nc.gpsimd.dma_start(
                out_ap[b, row0: row0 + P, eh * EB:(eh + 1) * EB], ot[:])

    def emit_pv(st):
        pT = st[5] if len(st) > 5 else pv_prep(st)
        pv_mms(st, pT)

    # ---- q loads: DMAs may be issued ahead (prefetched) of the cast
    def emit_qin_dmas(b, qb, qts):
        tiles = []
        for qt in qts:
            qin = qio.tile([P, E], f32, tag="qin", name="qin")
            nc.sync.dma_start(
                qin[:], q_ap[b, qb * QB + qt * P: qb * QB + (qt + 1) * P, :])
            tiles.append(qin)
        return tiles

    def emit_q16(qins):
        outs = []
        for qin in qins:
            q16 = q16p.tile([P, E], f16, tag="q16", name="q16", bufs=5)
            nc.vector.tensor_copy(q16[:], qin[:])
            outs.append(q16)
        return outs

    def emit_block_qT(b, qb, pre16):
        qT = big.tile([P, EC, QB], f16, tag="qTh", name="qT")
        q16s = pre16 + emit_q16(
            emit_qin_dmas(b, qb, range(len(pre16), qt_per_b)))
        for qt in range(qt_per_b):
            nc.sync.dma_start_transpose(
                qT[:, :, qt * P:(qt + 1) * P], q16s[qt][:])
        return qT

    def emit_block_qw(qT):
        qwT = big.tile([P, FC * QB], f16, tag="qwTh", name="qwT")
        for fc in range(FC):
            ps = psum.tile([P, QB], f32, tag="ps", name="ps")
            for ec in range(EC):
                nc.tensor.matmul(
                    ps[:], wH[:, ec * E + fc * P: ec * E + (fc + 1) * P],
                    qT[:, ec, :], start=(ec == 0), stop=(ec == EC - 1))
            nc.scalar.copy(qwT[:, fc * QB:(fc + 1) * QB], ps[:])
        return qwT

    for b in range(Bl):
        if b == 0:
            # W first in the sync FIFO (the first PE work, block-0 qw,
            # needs it), then block-0 q, then the k stream; qw overlaps
            # the k DMAs, scores trickle in as kTh chunks land
            emit_w_load()
            start_k(0)
            qT = emit_block_qT(0, 0, [])
            emit_k_chunks(0, range(KC))
            qwT = emit_block_qw(qT)
            kTh = emit_kTh(0)
            q16_pre = []
        else:
            # k16[b] already streamed during batch b-1's tail; only the
            # kTh transposes happen here (overlapped with block-0 qw
            # and the two deferred PVs of batch b-1)
            qT = emit_block_qT(b, 0, q16_pre)
            q16_pre = []
            qwT = emit_block_qw(qT)
            kTh = emit_kTh(b)

        qin_pre = []
        for qb in range(nqb):
            if qb > 0:
                qT = emit_block_qT(b, qb, q16_pre)
                q16_pre = []
                qwT = emit_block_qw(qT)

            for qt in range(qt_per_b):
                row0 = qb * QB + qt * P
                # mask prefetch: in the sync queue before the next
                # block's q rows, consumed after this tile's exp
                mts = []
                for kb in range(nkb):
                    mt = mio.tile([P, KB], i32, tag="mask", name="mt",
                                  bufs=6)
                    nc.sync.dma_start(
                        mt[:], mask_ap[b, row0: row0 + P,
                                       kb * KB:(kb + 1) * KB])
                    mts.append(mt)

                # xbar-transpose the newest deferred tile's
                # probabilities now: its mask multiplies have finished
                # by the time the sync queue reaches this instruction
                if pending:
                    pv_prep(pending[-1])

                # stream the next batch's k chunks during the last two
                # blocks of this batch (2 chunks per tile)
                if b + 1 < Bl and qb >= nqb - 2:
                    slot = (qb - (nqb - 2)) * qt_per_b + qt
                    if slot == 0:
                        start_k(b + 1)
                    emit_k_chunks(b + 1, [2 * slot, 2 * slot + 1])

                sp = work.tile([P, KL], f32, tag="sp", name="sp", bufs=1)
                mx = small.tile([P, nkb], f32, tag="mx", name="mx")
                for kb in range(nkb):
                    ps_s = psum.tile([P, KB], f32, tag="ps", name="ps_s")
                    for fc in range(FC):
                        nc.tensor.matmul(
                            ps_s[:],
                            qwT[:, fc * QB + qt * P: fc * QB + (qt + 1) * P],
                            kTh[:, fc, kb * KB:(kb + 1) * KB],
                            start=(fc == 0), stop=(fc == FC - 1))
                    nc.scalar.copy(sp[:, kb * KB:(kb + 1) * KB], ps_s[:])
                    nc.vector.tensor_reduce(
                        mx[:, kb:kb + 1], sp[:, kb * KB:(kb + 1) * KB],
                        axis=AX.X, op=ALU.max)

                negm = small.tile([P, 1], f32, tag="negm", name="negm")
                nc.vector.tensor_reduce(negm[:], mx[:], axis=AX.X,
                                        op=ALU.max, negate=True)

                spb = work.tile([P, KL], f16, tag="spb", name="spb")
                zs = small.tile([P, nkb], f32, tag="zs", name="zs")
                for kb in range(nkb):
                    blk = slice(kb * KB, (kb + 1) * KB)
                    nc.scalar.activation(spb[:, blk], sp[:, blk], AF.Exp,
                                         bias=negm[:],
                                         accum_out=zs[:, kb:kb + 1])
                    nc.vector.scalar_tensor_tensor(
                        out=spb[:, blk], in0=mts[kb][:], scalar=1.0,
                        in1=spb[:, blk], op0=ALU.mult, op1=ALU.mult)
                z = small.tile([P, 1], f32, tag="z", name="z")
                nc.vector.tensor_reduce(z[:], zs[:], axis=AX.X, op=ALU.add)
                rz = small.tile([P, 1], f32, tag="rz", name="rz")
                nc.vector.reciprocal(rz[:], z[:])

                # PV deferred by two tiles so the xbar transpose is
                # never on the critical path
                if len(pending) > 1:
                    st = pending.pop(0)
                    pv_mms(st, st[5])
                pending.append([b, row0, spb, rz, k16_of[b]])

                # prefetch the next block's q rows (DMA a tile before
                # the fp16 cast, so neither the sync queue nor the DVE
                # stalls): the block head then has no input dependency
                nxt = (b, qb + 1) if qb + 1 < nqb else (
                    (b + 1, 0) if b + 1 < Bl else None)
                if qt == qt_per_b - 3 and nxt is not None:
                    qin_pre = emit_qin_dmas(nxt[0], nxt[1], range(4))
                if qt == qt_per_b - 2 and qin_pre:
                    q16_pre = emit_q16(qin_pre)
                    qin_pre = []

    for st in pending:
        if len(st) <= 5:
            pv_prep(st)
    while pending:
        st = pending.pop(0)
        pv_mms(st, st[5])


def build_program(Bl, Q, KL, E, QB=512):
    nc = bacc.Bacc("TRN2", target_bir_lowering=False, debug=False)
    k_t = nc.dram_tensor("k", [Bl, KL, E], dt.float32, kind="ExternalInput")
    q_t = nc.dram_tensor("q", [Bl, Q, E], dt.float32, kind="ExternalInput")
    w_t = nc.dram_tensor("W", [E, E], dt.float32, kind="ExternalInput")
    m_t = nc.dram_tensor("mask", [Bl, Q, KL], dt.int32, kind="ExternalInput")
    o_t = nc.dram_tensor("out", [Bl, Q, E], dt.float32, kind="ExternalOutput")
    with tile.TileContext(nc) as tc:
        with ExitStack() as ctx:
            emit_attention(ctx, tc, k_t.ap(), q_t.ap(), w_t.ap(), m_t.ap(),
                           o_t.ap(), Bl, Q, KL, E, QB=QB)
    nc.compile()
    return nc


def kernel(k: np.ndarray, q: np.ndarray, W: np.ndarray, mask: np.ndarray,
           **run_kwargs) -> np.ndarray:
    assert k.shape == (B, K_LEN, EMB) and q.shape == (B, Q_LEN, EMB)
    assert W.shape == (EMB, EMB) and mask.shape == (B, Q_LEN, K_LEN)
    Bl = B // N_CORES
    nc = build_program(Bl, Q_LEN, K_LEN, EMB)
    in_maps = []
    for c in range(N_CORES):
        sl = slice(c * Bl, (c + 1) * Bl)
        in_maps.append({
            "k": np.ascontiguousarray(k[sl], dtype=np.float32),
            "q": np.ascontiguousarray(q[sl], dtype=np.float32),
            "W": np.ascontiguousarray(W, dtype=np.float32),
            "mask": np.ascontiguousarray(mask[sl], dtype=np.int32),
        })
    res = run_bass_kernel_spmd(nc, in_maps, core_ids=list(range(N_CORES)),
                               **run_kwargs)
    out = np.concatenate([r["out"] for r in res.results], axis=0)
    if run_kwargs.get("trace"):
        kernel.last_exec_time_ns = res.exec_time_ns
        kernel.last_result = res
    return out


kernel.last_exec_time_ns = None
kernel.last_result = None


# revision 6
# speedup vs baseline: 1.0258x; 1.0258x over previous
"""Trainium2 Bass kernel for masked bi-linear attention.

Computes, for full inputs
    k:    [B, KL, E] f32
    q:    [B, Q,  E] f32
    W:    [E, E]     f32
    mask: [B, Q, KL] i32 (0/1)
the reference
    qw    = q @ W                      [B, Q, E]
    s     = qw @ k^T                   [B, Q, KL]
    p     = softmax(s, axis=-1) * mask
    out   = p @ k                      [B, Q, E]

Sharding: data-parallel over B across 8 NeuronCores (2 batches/core),
W replicated. Each core runs the same Bass program on its B-slice.

Precision: q/W/k in fp16 (11-bit mantissa) for all three matmuls;
softmax in fp32 on ACT/DVE; probabilities (in [0,1]) in fp16 for p @ k.

The PE is the bottleneck (~90% occupancy): 2560 N=512 matmuls/core
stream at 1 col/cycle.  All transposes (q^T, k^T, p^T) run on the DMA
xbar (dma_start_transpose, fp16) so the PE does only matmuls.  The
xbar-transpose ucode occupies its issuing engine for the whole
transfer, so the transposes are split across the two HWDGE rings:
  sync ring:   q/k/W/mask HBM loads + the per-tile pT transposes
  scalar ring: the qT and kTh transposes
Data paths:
  q:  HBM -> qin f32 -> [DVE] q16 fp16 -> [xbar] qT [P, EC, QB]
  k:  HBM -> kin f32 -> [DVE] k16 [P, KC, E] fp16 (also the PV moving
      operand; double-buffered) -> [xbar] kTh [P, EC, KL]
  p:  exp+mask -> spb fp16 -> [xbar] pT [P, KC, P]

Pipelining, per q-tile t: scores(t) [PE] -> row-max [DVE] -> pT
transpose of t-1 [xbar] -> exp+mask (t) [ACT/DVE] -> PV matmuls of t-2
[PE] -> 1/z scale + output DMA.  The next block's q DMAs/casts/
transposes and its qw matmuls are all emitted inside the current block
(qwT double-buffered), so block heads have no dependency stall; the
next batch's k chunks stream during the previous batch's last two
blocks so the batch hand-off costs only the 16 kTh xbar transposes
(overlapped with the two deferred PVs and the prefetched block-0 qw).
"""

import numpy as np

import concourse.bacc as bacc
import concourse.mybir as mybir
import concourse.tile as tile
from concourse.bass_utils import run_bass_kernel_spmd
from contextlib import ExitStack

dt = mybir.dt
AF = mybir.ActivationFunctionType
ALU = mybir.AluOpType
AX = mybir.AxisListType

P = 128

N_CORES = 8
B, Q_LEN, K_LEN, EMB = 16, 2048, 2048, 1024


def emit_attention(ctx, tc, k_ap, q_ap, w_ap, mask_ap, out_ap,
                   Bl, Q, KL, E, QB=512):
    """Emit the per-core attention program.

    k_ap [Bl, KL, E], q_ap [Bl, Q, E], w_ap [E, E], mask_ap [Bl, Q, KL],
    out_ap [Bl, Q, E].
    """
    nc = tc.nc
    f32, f16, i32 = dt.float32, dt.float16, dt.int32

    assert Q % QB == 0 and QB % P == 0 and KL % P == 0 and E % P == 0
    EC = E // P          # e (contraction for qw) chunks
    KC = KL // P         # k chunks
    FC = E // P          # f chunks (qw output tiles)
    nqb = Q // QB
    qt_per_b = QB // P
    KB = min(512, KL)    # score psum block (<= 1 bank)
    nkb = KL // KB
    EB = min(512, E)     # PV psum block
    neb = E // EB

    big = ctx.enter_context(tc.tile_pool(name="big", bufs=1))
    qio = ctx.enter_context(tc.tile_pool(name="qio", bufs=4))
    q16p = ctx.enter_context(tc.tile_pool(name="q16p", bufs=4))
    mio = ctx.enter_context(tc.tile_pool(name="mio", bufs=2))
    ptp = ctx.enter_context(tc.tile_pool(name="ptp", bufs=4))
    work = ctx.enter_context(tc.tile_pool(name="work", bufs=2))
    small = ctx.enter_context(tc.tile_pool(name="small", bufs=3))
    psum = ctx.enter_context(tc.tile_pool(name="psum", bufs=4, space="PSUM"))
    psum_o = ctx.enter_context(tc.tile_pool(name="psum_o", bufs=2, space="PSUM"))

    # ---- W: loaded once per core as fp16
    wH = big.tile([P, EC * E], f16, tag="wH")

    def emit_w_load():
        for ec in range(EC):
            win = qio.tile([P, E], f32, tag="qin", name="win")
            nc.sync.dma_start(win[:], w_ap[ec * P:(ec + 1) * P, :])
            nc.scalar.copy(wH[:, ec * E:(ec + 1) * E], win[:])

    # ---- k pipeline: kin f32 DMA -> k16 fp16 (DVE cast); the fp16
    # natural-layout tensor is both the PV moving operand and the xbar
    # transpose source for kTh
    k16_of = {}

    def start_k(b):
        t = big.tile([P, KC, E], f16, tag="k16", name="k16", bufs=2)
        k16_of[b] = t
        return t

    def emit_k_chunks(b, kcs):
        t = k16_of[b]
        for kc in kcs:
            kin = qio.tile([P, E], f32, tag="qin", name="kin")
            nc.sync.dma_start(kin[:], k_ap[b, kc * P:(kc + 1) * P, :])
            nc.vector.tensor_copy(t[:, kc, :], kin[:])

    def emit_kTh(b):
        # all xbar transposes share the sync ring: the transpose ucode
        # occupies the issuing engine, and two rings running transposes
        # concurrently corrupt data (Tile does not guard that pair)
        kTh = big.tile([P, EC, KL], f16, tag="kTh", name="kTh")
        t = k16_of[b]
        for kc in range(KC):
            nc.sync.dma_start_transpose(
                kTh[:, :, kc * P:(kc + 1) * P], t[:, kc, :])
        return kTh

    # deferred-PV state: (b, row0, spb, rz, k16t, [pT])
    pending = []

    def pv_prep(st):
        # p^T via the DMA xbar (SBUF->SBUF, fp16) on the sync queue
        spb = st[2]
        pT = ptp.tile([P, KC, P], f16, tag="pT", name="pT", bufs=2)
        nc.sync.dma_start_transpose(pT[:], spb[:])
        st.append(pT)
        return pT

    def pv_mms(st, pT):
        b, row0, spb, rz, k16t = st[:5]
        po = [psum_o.tile([P, EB], f32, tag=f"po{eh}", name=f"po{eh}")
              for eh in range(neb)]
        for kc in range(KC):
            for eh in range(neb):
                nc.tensor.matmul(
                    po[eh][:], pT[:, kc, :],
                    k16t[:, kc, eh * EB:(eh + 1) * EB],
                    start=(kc == 0), stop=(kc == KC - 1))
        for eh in range(neb):
            ot = mio.tile([P, EB], f32, tag="ot", name="ot")
            nc.scalar.activation(ot[:], po[eh][:], AF.Copy, scale=rz[:])
            nc.gpsimd.dma_start(
                out_ap[b, row0: row0 + P, eh * EB:(eh + 1) * EB], ot[:])

    # ---- q pipeline
    def emit_qin_dmas(b, qb, qts):
        tiles = []
        for qt in qts:
            qin = qio.tile([P, E], f32, tag="qin", name="qin")
            nc.sync.dma_start(
                qin[:], q_ap[b, qb * QB + qt * P: qb * QB + (qt + 1) * P, :])
            tiles.append(qin)
        return tiles

    def emit_q16(qins):
        outs = []
        for qin in qins:
            q16 = q16p.tile([P, E], f16, tag="q16", name="q16", bufs=4)
            nc.vector.tensor_copy(q16[:], qin[:])
            outs.append(q16)
        return outs

    def emit_qT(q16s):
        qT = big.tile([P, EC, QB], f16, tag="qTh", name="qT")
        for qt in range(qt_per_b):
            nc.sync.dma_start_transpose(
                qT[:, :, qt * P:(qt + 1) * P], q16s[qt][:])
        return qT

    def emit_block_qw(qT):
        qwT = big.tile([P, FC * QB], f16, tag="qwTh", name="qwT", bufs=2)
        for fc in range(FC):
            ps = psum.tile([P, QB], f32, tag="ps", name="ps")
            for ec in range(EC):
                nc.tensor.matmul(
                    ps[:], wH[:, ec * E + fc * P: ec * E + (fc + 1) * P],
                    qT[:, ec, :], start=(ec == 0), stop=(ec == EC - 1))
            nc.scalar.copy(qwT[:, fc * QB:(fc + 1) * QB], ps[:])
        return qwT

    qwT_next = None
    qin_pre, q16_pre = [], []

    for b in range(Bl):
        if b == 0:
            # W first in the sync FIFO (the first PE work, block-0 qw,
            # needs it), then block-0 q and the first k chunks; the qT
            # transpose ucodes stall the ring head waiting on the q
            # casts, so the first k chunks are issued before them and
            # stream meanwhile; qw overlaps the rest of the k DMAs and
            # scores trickle in as kTh chunks land
            emit_w_load()
            start_k(0)
            q16s = emit_q16(emit_qin_dmas(0, 0, range(qt_per_b)))
            emit_k_chunks(0, range(4))
            qT = emit_qT(q16s)
            emit_k_chunks(0, range(4, KC))
            qwT_next = emit_block_qw(qT)
        # k16[b] is fully streamed by now (during batch b-1's tail for
        # b > 0); the kTh transposes chase the last batch's final
        # scores chunk-by-chunk on the scalar ring
        kTh = emit_kTh(b)

        for qb in range(nqb):
            qwT = qwT_next

            for qt in range(qt_per_b):
                row0 = qb * QB + qt * P
                # mask prefetch: one [P, KL] DMA per tile on the sync
                # ring, consumed after this tile's exp
                mt = mio.tile([P, KL], i32, tag="mask", name="mt",
                              bufs=2)
                nc.sync.dma_start(mt[:], mask_ap[b, row0: row0 + P, :])

                # xbar-transpose the newest deferred tile's
                # probabilities now: its mask multiplies have finished
                # by the time the sync queue reaches this instruction
                if pending:
                    pv_prep(pending[-1])

                # stream the next batch's k chunks during the last two
                # blocks of this batch (2 chunks per tile)
                if b + 1 < Bl and qb >= nqb - 2:
                    slot = (qb - (nqb - 2)) * qt_per_b + qt
                    if slot == 0:
                        start_k(b + 1)
                    emit_k_chunks(b + 1, [2 * slot, 2 * slot + 1])

                # next block's q pipeline, one block ahead: DMAs at
                # qt0, casts at qt1, qT transposes + qw matmuls at qt2
                nxt = (b, qb + 1) if qb + 1 < nqb else (
                    (b + 1, 0) if b + 1 < Bl else None)
                if nxt is not None:
                    if qt == 0:
                        qin_pre = emit_qin_dmas(nxt[0], nxt[1],
                                                range(qt_per_b))
                    elif qt == 1:
                        q16_pre = emit_q16(qin_pre)
                        qin_pre = []
                    elif qt == 2:
                        qwT_next = emit_block_qw(emit_qT(q16_pre))
                        q16_pre = []

                sp = work.tile([P, KL], f32, tag="sp", name="sp", bufs=1)
                mx = small.tile([P, nkb], f32, tag="mx", name="mx")
                for kb in range(nkb):
                    ps_s = psum.tile([P, KB], f32, tag="ps", name="ps_s")
                    for fc in range(FC):
                        nc.tensor.matmul(
                            ps_s[:],
                            qwT[:, fc * QB + qt * P: fc * QB + (qt + 1) * P],
                            kTh[:, fc, kb * KB:(kb + 1) * KB],
                            start=(fc == 0), stop=(fc == FC - 1))
                    nc.scalar.copy(sp[:, kb * KB:(kb + 1) * KB], ps_s[:])
                    nc.vector.tensor_reduce(
                        mx[:, kb:kb + 1], sp[:, kb * KB:(kb + 1) * KB],
                        axis=AX.X, op=ALU.max)

                negm = small.tile([P, 1], f32, tag="negm", name="negm")
                nc.vector.tensor_reduce(negm[:], mx[:], axis=AX.X,
                                        op=ALU.max, negate=True)

                spb = work.tile([P, KL], f16, tag="spb", name="spb")
                zs = small.tile([P, nkb], f32, tag="zs", name="zs")
                for kb in range(nkb):
                    blk = slice(kb * KB, (kb + 1) * KB)
                    nc.scalar.activation(spb[:, blk], sp[:, blk], AF.Exp,
                                         bias=negm[:],
                                         accum_out=zs[:, kb:kb + 1])
                    nc.vector.scalar_tensor_tensor(
                        out=spb[:, blk], in0=mt[:, blk], scalar=1.0,
                        in1=spb[:, blk], op0=ALU.mult, op1=ALU.mult)
                z = small.tile([P, 1], f32, tag="z", name="z")
                nc.vector.tensor_reduce(z[:], zs[:], axis=AX.X, op=ALU.add)
                rz = small.tile([P, 1], f32, tag="rz", name="rz")
                nc.vector.reciprocal(rz[:], z[:])

                # PV deferred by two tiles so the xbar transpose is
                # never on the critical path
                if len(pending) > 1:
                    st = pending.pop(0)
                    pv_mms(st, st[5])
                pending.append([b, row0, spb, rz, k16_of[b]])

    for st in pending:
        if len(st) <= 5:
            pv_prep(st)
    while pending:
        st = pending.pop(0)
        pv_mms(st, st[5])


def build_program(Bl, Q, KL, E, QB=512):
    nc = bacc.Bacc("TRN2", target_bir_lowering=False, debug=False)
    k_t = nc.dram_tensor("k", [Bl, KL, E], dt.float32, kind="ExternalInput")
    q_t = nc.dram_tensor("q", [Bl, Q, E], dt.float32, kind="ExternalInput")
    w_t = nc.dram_tensor("W", [E, E], dt.float32, kind="ExternalInput")
    m_t = nc.dram_tensor("mask", [Bl, Q, KL], dt.int32, kind="ExternalInput")
    o_t = nc.dram_tensor("out", [Bl, Q, E], dt.float32, kind="ExternalOutput")
    with tile.TileContext(nc) as tc:
        with ExitStack() as ctx:
            emit_attention(ctx, tc, k_t.ap(), q_t.ap(), w_t.ap(), m_t.ap(),
                           o_t.ap(), Bl, Q, KL, E, QB=QB)
    nc.compile()
    return nc


def kernel(k: np.ndarray, q: np.ndarray, W: np.ndarray, mask: np.ndarray,
           **run_kwargs) -> np.ndarray:
    assert k.shape == (B, K_LEN, EMB) and q.shape == (B, Q_LEN, EMB)
    assert W.shape == (EMB, EMB) and mask.shape == (B, Q_LEN, K_LEN)
    Bl = B // N_CORES
    nc = build_program(Bl, Q_LEN, K_LEN, EMB)
    in_maps = []
    for c in range(N_CORES):
        sl = slice(c * Bl, (c + 1) * Bl)
        in_maps.append({
            "k": np.ascontiguousarray(k[sl], dtype=np.float32),
            "q": np.ascontiguousarray(q[sl], dtype=np.float32),
            "W": np.ascontiguousarray(W, dtype=np.float32),
            "mask": np.ascontiguousarray(mask[sl], dtype=np.int32),
        })
    res = run_bass_kernel_spmd(nc, in_maps, core_ids=list(range(N_CORES)),
                               **run_kwargs)
    out = np.concatenate([r["out"] for r in res.results], axis=0)
    if run_kwargs.get("trace"):
        kernel.last_exec_time_ns = res.exec_time_ns
        kernel.last_result = res
    return out


kernel.last_exec_time_ns = None
kernel.last_result = None
